# revision 39
# baseline (speedup 1.0000x reference)
"""Trainium2 Bass kernel for nn_AttentionBlock (GroupNorm + 4-head attention + proj + residual).

Problem (hardcoded): x [4, 256, 64, 64] f32, 32 groups, 4 heads (ch=64/head),
T = 64*64 = 4096 tokens per batch item.

Sharding over 8 NeuronCores: core = (batch b, query-half). Each core receives
x[b] with its token-half rotated to the front, computes GroupNorm(x[b]) + full
K/V for all 4 heads, attention for its 2048 queries, proj + residual. Outputs
are disjoint -> host stitches halves. No collectives.

v2 engine plan (from HW microbenchmarks):
 - ST (q@k): bf16 row-tiled concurrent pairs (two K=64 matmuls in row groups
   0-1 / 2-3 run concurrently, ~222 ns per 128-key block for both heads).
 - softmax exp: split across ACT and DVE by key-block PAIR. ACT pairs:
   Exp -> fp8e4 with a -4 bias shift (so e^<=2 fits e4m3). DVE pairs:
   Schraudolph bit-trick, tensor_scalar (x*A+B) -> int8 = fp8e5 bit pattern
   (same -4 shift folded into B). Both paths feed the same PV accumulation,
   multiplicative exp error cancels in the Z normalization.
 - PV: fp8 DoubleRow over key-block pairs: lhsT = vT fp8e4 [128, 2, 65]
   (65th col = ones -> Z row), rhs = pt [128, 2, 512] (e4 or e5-bitcast),
   ~215 ns per s-pair per head at 2.4 GHz (2x over bf16).
 - normalize: reciprocal_approx_fast on the PSUM Z row, f32r ones-matmul
   broadcast into the unused partitions 64..127 of the same PV psum bank,
   ACT copies it to SBUF, one DVE scalar_tensor_tensor does a = pv * rzb.
 - v bias folded through attention (sum p = Z) into Wp@bv, precomputed by PE
   and added during output staging on GpSimd (which also applies GroupNorm).
"""

import functools
import os

import numpy as np

import concourse.bass as bass
import concourse.tile as tile
from concourse import bacc, mybir
from concourse.bass_utils import run_bass_kernel_spmd

F32 = mybir.dt.float32
F32R = mybir.dt.float32r
BF16 = mybir.dt.bfloat16
FP8E4 = mybir.dt.float8e4
FP8E5 = mybir.dt.float8e5
I8 = mybir.dt.int8
AF = mybir.ActivationFunctionType
OP = mybir.AluOpType
DR = mybir.MatmulPerfMode.DoubleRow

P = 128          # partitions
C = 256          # channels
T = 4096         # tokens per batch item
TQ = 2048        # query tokens per core (half of T)
GS = 8           # channels per group
EPS = 1e-5
SCALE = 0.125    # 1/sqrt(ch) applied inside exp
# No exp shift: pt is fp8e5 on both paths, whose range [2^-16, 57344] covers
# e^u for u in [-11, 10.9] directly. A negative shift would push the DVE
# trick's int8 bits negative for rare ~-6.5 sigma scores (0xFF = e5m2 NaN).
SHIFT = 0.0
N_CORES = 8

LOG2E = 1.4426950408889634
A5 = SCALE * LOG2E * 4.0                  # e5m2 trick: bits = s*A5 + B5
B5 = 60.0 - 0.15 + SHIFT * LOG2E * 4.0

# key-block steps (0..31) whose exp runs on DVE (rest on ACT). Per-STEP (not
# per-pair) assignment lets the two engines drain the two st psum slots
# concurrently. Bresenham-spread KDVE_N of 32 steps onto DVE.
KDVE_N = int(os.environ.get("KDVE_N", "13"))
KDVE_N_STRIPE = int(os.environ.get("KDVE_N_STRIPE", "12"))
DVE_S = frozenset(
    s for s in range(32) if (s * KDVE_N) // 32 != ((s + 1) * KDVE_N) // 32)


def _emit(nc, tc, aps):
    xp, wqkv, bqkv, wproj, bproj, gamma, beta, outp, dbgo = aps

    with (
        tc.tile_pool(name="pp", bufs=1) as pp,
        tc.tile_pool(name="wk", bufs=3) as wk,
        tc.tile_pool(name="ptp", bufs=7) as ptp,
        tc.tile_pool(name="ps", bufs=3, space="PSUM") as ps,
    ):
        # ---------------- constants (NEFF-embedded) ----------------
        import ml_dtypes
        ident_np = np.eye(P, dtype=ml_dtypes.bfloat16)
        selg_np = (np.arange(P)[:, None] // GS == np.arange(16)[None, :]).astype(np.float32)
        ident_dram = nc.inline_tensor(ident_np, name="ident_c")
        selg_dram = nc.inline_tensor(selg_np, name="selg_c")
        selgT_dram = nc.inline_tensor(np.ascontiguousarray(selg_np.T), name="selgT_c")

        ident = pp.tile([P, P], BF16)
        nc.sync.dma_start(out=ident, in_=ident_dram.ap())
        selg = pp.tile([P, 16], F32)
        nc.sync.dma_start(out=selg, in_=selg_dram.ap())
        selgT = pp.tile([16, P], F32)
        nc.sync.dma_start(out=selgT, in_=selgT_dram.ap())

        eps16 = pp.tile([16, 1], F32)
        nc.vector.memset(eps16, EPS)
        ones64 = pp.tile([1, 64], F32)
        nc.vector.memset(ones64, 1.0)
        ones64b = pp.tile([1, 64], BF16)
        nc.vector.memset(ones64b, 1.0)
        m4 = pp.tile([P, 1], F32)
        nc.vector.memset(m4, SHIFT)

        # ---------------- input DMAs ----------------
        x_sb = [pp.tile([P, T], F32, name=f"x_sb{ct}") for ct in range(2)]
        for chk in range(8):
            for ct in range(2):
                nc.sync.dma_start(
                    out=x_sb[ct][:, 512 * chk:512 * (chk + 1)],
                    in_=xp[P * ct:P * (ct + 1), 512 * chk:512 * (chk + 1)])

        wq_st = pp.tile([P, 6, C], F32)
        nc.sync.dma_start(out=wq_st, in_=wqkv.rearrange("(a p) c -> p a c", p=P))
        wp_st = pp.tile([P, 2, C], F32)
        nc.sync.dma_start(out=wp_st, in_=wproj.rearrange("(a p) c -> p a c", p=P))

        bq = [pp.tile([P, 1], F32, name=f"bq{p_}") for p_ in range(2)]
        bk = [pp.tile([P, 1], F32, name=f"bk{p_}") for p_ in range(2)]
        bva = [pp.tile([P, 1], F32, name=f"bva{p_}") for p_ in range(2)]
        for pr in range(2):
            for a in range(2):
                h = 2 * pr + a
                sl = slice(64 * a, 64 * (a + 1))
                nc.sync.dma_start(out=bq[pr][sl, :], in_=bqkv[192 * h:192 * h + 64, :])
                nc.sync.dma_start(out=bk[pr][sl, :], in_=bqkv[192 * h + 64:192 * h + 128, :])
                nc.sync.dma_start(out=bva[pr][sl, :], in_=bqkv[192 * h + 128:192 * h + 192, :])
        gam = [pp.tile([P, 1], F32, name=f"gam{ct}") for ct in range(2)]
        bet = [pp.tile([P, 1], F32, name=f"bet{ct}") for ct in range(2)]
        bp = [pp.tile([P, 1], F32, name=f"bp{ct}") for ct in range(2)]
        for ct in range(2):
            sl = slice(P * ct, P * (ct + 1))
            nc.sync.dma_start(out=gam[ct], in_=gamma[sl, :])
            nc.sync.dma_start(out=bet[ct], in_=beta[sl, :])
            nc.sync.dma_start(out=bp[ct], in_=bproj[sl, :])

        # ---------------- GroupNorm statistics ----------------
        rhs_stats = pp.tile([P, 4], F32)
        for ct in range(2):
            xv = x_sb[ct].rearrange("p (n f) -> p n f", f=512)
            stats = wk.tile([P, 8, 6], F32, tag="bnstats")
            for i in range(8):
                nc.vector.bn_stats(out=stats[:, i, :], in_=xv[:, i, :])
            mv = wk.tile([P, 2], F32, tag="bnmv")
            nc.vector.bn_aggr(out=mv, in_=stats)
            nc.vector.tensor_copy(out=rhs_stats[:, 2 * ct:2 * ct + 1], in_=mv[:, 0:1])
            m2 = wk.tile([P, 1], F32, tag="m2")
            nc.vector.tensor_mul(out=m2, in0=mv[:, 0:1], in1=mv[:, 0:1])
            nc.vector.tensor_add(out=rhs_stats[:, 2 * ct + 1:2 * ct + 2],
                                 in0=mv[:, 1:2], in1=m2)

        gst_ps = ps.tile([16, 4], F32, tag="st")
        nc.tensor.matmul(out=gst_ps, lhsT=selg, rhs=rhs_stats, start=True, stop=True)
        gstat = pp.tile([16, 4], F32)
        nc.vector.tensor_scalar_mul(out=gstat, in0=gst_ps, scalar1=1.0 / GS)
        g3 = gstat.rearrange("p (a b) -> p a b", b=2)
        mu2 = pp.tile([16, 2], F32)
        nc.vector.tensor_mul(out=mu2, in0=g3[:, :, 0], in1=g3[:, :, 0])
        var_t = pp.tile([16, 2], F32)
        nc.vector.tensor_sub(out=var_t, in0=g3[:, :, 1], in1=mu2)
        std_t = pp.tile([16, 2], F32)
        nc.scalar.activation(out=std_t, in_=var_t, func=AF.Sqrt, bias=eps16, scale=1.0)
        expwarm = wk.tile([P, 1], F32, tag="expwarm")
        nc.scalar.activation(out=expwarm, in_=m4, func=AF.Exp, scale=SCALE, bias=m4)
        rs_t = pp.tile([16, 2], F32)
        nc.vector.reciprocal(out=rs_t, in_=std_t)

        A_t = [pp.tile([P, 1], F32, name=f"A_t{ct}") for ct in range(2)]
        B_t = [pp.tile([P, 1], F32, name=f"B_t{ct}") for ct in range(2)]
        for ct in range(2):
            rhs_bc = wk.tile([16, 2], F32, tag="rhsbc")
            nc.vector.tensor_copy(out=rhs_bc[:, 0:1], in_=gstat[:, 2 * ct:2 * ct + 1])
            nc.vector.tensor_copy(out=rhs_bc[:, 1:2], in_=rs_t[:, ct:ct + 1])
            bc_ps = ps.tile([P, 2], F32, tag="st", name=f"bc_ps{ct}")
            nc.tensor.matmul(out=bc_ps, lhsT=selgT, rhs=rhs_bc, start=True, stop=True)
            nc.vector.tensor_mul(out=A_t[ct], in0=bc_ps[:, 1:2], in1=gam[ct])
            tb = wk.tile([P, 1], F32, tag="tb")
            nc.vector.tensor_mul(out=tb, in0=bc_ps[:, 0:1], in1=A_t[ct])
            nc.vector.tensor_sub(out=B_t[ct], in0=bet[ct], in1=tb)

        # h = x * A + B: fp8e4 in DoubleRow rhs layout [128, 2(ct), T] (DVE),
        # feeding K=256 DoubleRow QKV matmuls in one pass.
        h8 = pp.tile([P, 2, T], FP8E4, name="h8")
        for chk in range(4):
            csl = slice(1024 * chk, 1024 * (chk + 1))
            for ct in range(2):
                nc.vector.tensor_scalar(out=h8[:, ct, csl], in0=x_sb[ct][:, csl],
                                        scalar1=A_t[ct], scalar2=B_t[ct],
                                        op0=OP.mult, op1=OP.add)

        # ---------------- weight transposes (PE identity matmuls) ----------------
        wq_bf = pp.tile([P, 6, C], BF16)
        nc.scalar.copy(out=wq_bf, in_=wq_st)
        wp_bf = pp.tile([P, 2, C], BF16)
        nc.scalar.copy(out=wp_bf, in_=wp_st)
        WTq8 = pp.tile([P, 2, 768], FP8E4, name="WTq8")
        WTp = [pp.tile([P, C], BF16, name=f"WTp{j}") for j in range(2)]
        for i in range(6):
            for j in range(2):
                tq_ps = ps.tile([P, P], BF16, tag="st", name=f"tq_ps{i}{j}")
                nc.tensor.transpose(out=tq_ps, in_=wq_bf[:, i, P * j:P * (j + 1)],
                                    identity=ident)
                for a in range(2):
                    o0 = P * i + 64 * a
                    h = o0 // 192
                    kind = (o0 % 192) // 64
                    dcol = kind * 256 + (h // 2) * 128 + (h % 2) * 64
                    nc.scalar.copy(out=WTq8[:, j, dcol:dcol + 64],
                                   in_=tq_ps[:, 64 * a:64 * (a + 1)])
        for i in range(2):
            for j in range(2):
                tp_ps = ps.tile([P, P], BF16, tag="st", name=f"tp_ps{i}{j}")
                nc.tensor.transpose(out=tp_ps, in_=wp_bf[:, i, P * j:P * (j + 1)],
                                    identity=ident)
                nc.scalar.copy(out=WTp[j][:, P * i:P * (i + 1)], in_=tp_ps)

        def wt8_slice(kind, pr):
            base = kind * 256 + pr * 128
            return WTq8[:, :, base:base + 128]

        # c = Wp @ bv  (v-bias passes through attention; fold into out staging)
        bva_bf = [pp.tile([P, 1], BF16, name=f"bvab{j}") for j in range(2)]
        for j in range(2):
            nc.scalar.copy(out=bva_bf[j], in_=bva[j])
        c_sb = [pp.tile([P, 1], F32, name=f"c_sb{oc}") for oc in range(2)]
        for oc in range(2):
            c_ps = ps.tile([P, 1], F32, tag="st", name=f"c_ps{oc}")
            for j in range(2):
                nc.tensor.matmul(out=c_ps, lhsT=WTp[j][:, P * oc:P * (oc + 1)],
                                 rhs=bva_bf[j], start=(j == 0), stop=(j == 1))
            nc.vector.tensor_copy(out=c_sb[oc], in_=c_ps)

        # out staging: out_sb = x + (proj_bias + Wp@bv)   (ACT Identity)
        out_sb = [pp.tile([P, TQ], F32, name=f"out_sb{ct}") for ct in range(2)]
        bpc = [pp.tile([P, 1], F32, name=f"bpc{ct}") for ct in range(2)]
        for ct in range(2):
            nc.vector.tensor_add(out=bpc[ct], in0=bp[ct], in1=c_sb[ct])
            for hk in range(2):
                qsl = slice(1024 * hk, 1024 * (hk + 1))
                nc.scalar.activation(out=out_sb[ct][:, qsl], in_=x_sb[ct][:, qsl],
                                     func=AF.Identity, bias=bpc[ct], scale=1.0)

        # ---------------- QKV projections ----------------
        q_sb = [pp.tile([P, TQ], BF16, name=f"q_sb{p_}") for p_ in range(2)]
        k_sb = [pp.tile([P, T], BF16, name=f"k_sb{p_}") for p_ in range(2)]
        # vT fp8e4 DoubleRow layout: [keys-in-block, s-pair, parity, head, 80]
        # (col 64 = ones -> Z row; cols 65..79 pad for 16B-aligned strides)
        vT8 = [pp.tile([P, 16, 2, 2, 80], FP8E4, name=f"vT8{p_}") for p_ in range(2)]

        def _qkv_stripe_parts(pr, tt):
            # One 512-column stripe: k (+q for tt<4) and vT for pair pr.
            if tt == 0:
                nc.vector.memset(
                    vT8[pr].rearrange("p sp par h c -> p (sp par h) c")[:, :, 64:65],
                    1.0)
            tsl = slice(512 * tt, 512 * (tt + 1))
            s1 = ps.tile([P, 2, 512], F32, tag="st", name=f"qk_ps{pr}{tt}")
            nc.tensor.matmul(out=s1[:, 0, :], lhsT=wt8_slice(1, pr),
                             rhs=h8[:, :, tsl], start=True, stop=True,
                             perf_mode=DR)
            nc.vector.tensor_scalar_add(out=k_sb[pr][:, tsl], in0=s1[:, 0, :],
                                        scalar1=bk[pr])
            if tt < 4:
                nc.tensor.matmul(out=s1[:, 1, :], lhsT=wt8_slice(0, pr),
                                 rhs=h8[:, :, tsl], start=True, stop=True,
                                 perf_mode=DR)
                nc.vector.tensor_scalar_add(out=q_sb[pr][:, tsl],
                                            in0=s1[:, 1, :], scalar1=bq[pr])
                yield
                vtile = ps.tile([P, 2, 512], F32, tag="st", name=f"v_ps{pr}{tt}")
                vsl = vtile[:, 0, :]
            else:
                yield
                vsl = s1[:, 1, :]
            vv = vsl.rearrange("p (j n) -> p j n", j=4)
            for j in range(4):
                it = 4 * tt + j
                nc.tensor.matmul(out=vv[:, j, :],
                                 lhsT=h8[:, :, P * it:P * (it + 1)],
                                 rhs=wt8_slice(2, pr),
                                 start=True, stop=True, perf_mode=DR)
            # cast v to fp8e4 into the DoubleRow layout (one DVE op per stripe)
            nc.vector.tensor_copy(
                out=vT8[pr].rearrange("p sp par h c -> p (sp par) h c")[
                    :, 4 * tt:4 * (tt + 1), :, 0:64],
                in_=vsl.rearrange("p (j h c) -> p j h c", j=4, h=2))
            yield

        def emit_qkv_stripe(pr, tt):
            for _ in _qkv_stripe_parts(pr, tt):
                pass

        def gen_qkv_stripe(pr, tt):
            return iter(_qkv_stripe_parts(pr, tt))

        # ---------------- attention ----------------
        kstage = int(os.environ.get("KSTAGE", "9"))
        a_sb = [pp.tile([P, TQ], BF16, name=f"a_sb{p_}") for p_ in range(2)]
        if kstage <= 3:
            for pr_ in range(2):
                nc.vector.memset(a_sb[pr_], 0.0)
        n_tt = 0 if kstage <= 2 else (1 if kstage == 3 else 4)

        gens_pv = {}

        def gen_attn(pr, tt, dve_s=DVE_S):
            """Pipeline-step generator for one (pair, q-tile). Step s<32 emits
            the row-tiled ST pair + exp (engine by s-pair); odd steps >=3 emit
            the fp8 DoubleRow PV pair for s-pair (s-3)//2; steps 32..34 drain
            the last PVs."""
            tsl = slice(512 * tt, 512 * (tt + 1))
            pvp = ps.tile([P, 2, 512], F32, tag="pv", bufs=1, name=f"pv{pr}{tt}")
            gens_pv[(pr, tt)] = pvp
            pts = {}
            for s in range(39):
                if s < 32:
                    sp, par = s // 2, s % 2
                    st = ps.tile([P, 2, 512], F32, tag="st", bufs=3,
                                 name=f"st{pr}{tt}{s}")
                    for h in range(2):
                        nc.tensor.matmul(
                            out=st[:, h, :],
                            lhsT=k_sb[pr][64 * h:64 * (h + 1), P * s:P * (s + 1)],
                            rhs=q_sb[pr][64 * h:64 * (h + 1), tsl],
                            start=True, stop=True)
                    # pt is fp8e5 on both engines (e4m3 would overflow to inf
                    # on rare >9.5-sigma diagonal scores).
                    if par == 0:
                        pts[sp] = ptp.tile([P, 2, 2, 512], FP8E5, tag="pt",
                                           name=f"pt{pr}{tt}{sp}")
                    if s in dve_s:
                        nc.vector.tensor_scalar(
                            out=pts[sp][:, par, :, :].bitcast(I8), in0=st,
                            scalar1=A5, scalar2=B5, op0=OP.mult, op1=OP.add)
                    else:
                        nc.scalar.activation(out=pts[sp][:, par, :, :], in_=st,
                                             func=AF.Exp, scale=SCALE, bias=m4)
                if s == 8 or (s >= 9 and s % 2 == 1):
                    sps = [0] if s == 8 else [(s - 7) // 2]
                    for sp_i in sps:
                        pt = pts.pop(sp_i)
                        for h in range(2):
                            nc.tensor.matmul(out=pvp[0:65, h, :],
                                             lhsT=vT8[pr][:, sp_i, :, h, 0:65],
                                             rhs=pt[:, :, h, :],
                                             start=(sp_i == 0), stop=(sp_i == 15),
                                             perf_mode=DR)
                yield

        kdbg = os.environ.get("KDBG", "") == "1"
        if kdbg:
            dbg_sb = pp.tile([P, 3, 512], F32, name="dbg_sb")

        def gen_normalize(pr, tt):
            # a = pv[0:64] * (1/Z); Z in row 64 of each head's bank. Yields
            # between phases so the ops interleave with the next tile's steps
            # instead of monopolizing the engines at the boundary.
            pvp = gens_pv.pop((pr, tt))
            tsl = slice(512 * tt, 512 * (tt + 1))
            zrow2 = wk.tile([1, 2, 512], F32, tag="zrow")
            nc.vector.tensor_copy(out=zrow2, in_=pvp[64:65, :, :])
            yield
            rz2 = wk.tile([1, 2, 512], F32, tag="rz")
            nc.vector.reciprocal_approx_fast(out=rz2, in_=zrow2)
            yield
            rzbf = wk.tile([1, 2, 512], BF16, tag="rzbf")
            nc.vector.tensor_copy(out=rzbf, in_=rz2)
            yield
            for h in range(2):
                nc.tensor.matmul(out=pvp[64:128, h, :],
                                 lhsT=ones64b, rhs=rzbf[:, h, :],
                                 start=True, stop=True)
            yield
            rzb = wk.tile([64, 2, 512], F32, tag="rzb")
            nc.scalar.copy(out=rzb[:, 0, :], in_=pvp[64:128, 0, :])
            yield
            nc.scalar.copy(out=rzb[:, 1, :], in_=pvp[64:128, 1, :])
            yield
            nc.vector.scalar_tensor_tensor(
                out=a_sb[pr][0:64, tsl],
                in0=pvp[0:64, 0, :], scalar=1.0, in1=rzb[:, 0, :],
                op0=OP.mult, op1=OP.mult)
            yield
            nc.vector.scalar_tensor_tensor(
                out=a_sb[pr][64:128, tsl],
                in0=pvp[0:64, 1, :], scalar=1.0, in1=rzb[:, 1, :],
                op0=OP.mult, op1=OP.mult)

        proj_done = set()

        def gen_proj_tt(tt):
            # proj + residual for one 512-column stripe; psums borrow st slots.
            proj_done.add(tt)
            tsl = slice(512 * tt, 512 * (tt + 1))
            for oc in range(2):
                pj = ps.tile([P, 512], F32, tag="st", bufs=3, name=f"pj{oc}{tt}")
                for ct in range(2):
                    nc.tensor.matmul(out=pj, lhsT=WTp[ct][:, P * oc:P * (oc + 1)],
                                     rhs=a_sb[ct][:, tsl],
                                     start=(ct == 0), stop=(ct == 1))
                yield
                nc.vector.tensor_add(out=out_sb[oc][:, tsl],
                                     in0=out_sb[oc][:, tsl], in1=pj)
                nc.sync.dma_start(out=outp[P * oc:P * (oc + 1), tsl],
                                  in_=out_sb[oc][:, tsl])
                yield

        def drive(g, n):
            for _ in range(n):
                next(g, None)

        # ---- flat schedule: qkv stripes + normalize/proj interleaved ----
        emit_qkv_stripe(0, 0)
        tiles = [(0, t_) for t_ in range(n_tt)]
        if kstage >= 4:
            tiles += [(1, t_) for t_ in range(n_tt)]
        prev = None
        for idx, (pr, tt) in enumerate(tiles):
            ndve = KDVE_N_STRIPE if (pr, tt) in ((0, 0), (0, 2), (0, 3)) else KDVE_N
            dve_s = frozenset(
                s_ for s_ in range(32)
                if (s_ * ndve) // 32 != ((s_ + 1) * ndve) // 32)
            g = gen_attn(pr, tt, dve_s)
            aux = []
            if prev is not None:
                import itertools as _it
                if prev[0] == 1:
                    aux.append(_it.chain(gen_normalize(*prev),
                                         gen_proj_tt(prev[1])))
                else:
                    aux.append(gen_normalize(*prev))
            if idx == 0:
                for i in range(1, 8):
                    sp_ = gen_qkv_stripe(0, i)
                    drive(g, 2)
                    next(sp_, None)
                    drive(g, 2)
                    next(sp_, None)
                    next(sp_, None)
                drive(g, 11)
            elif kstage >= 4 and (pr, tt) in ((0, 2), (0, 3)):
                base = 0 if tt == 2 else 4
                done = 0
                for si in range(base, base + 4):
                    sp_ = gen_qkv_stripe(1, si)
                    mid = min(8 * (si - base) + 5, 39)
                    while done < mid:
                        drive(g, 1)
                        done += 1
                        for a_ in aux:
                            next(a_, None)
                    next(sp_, None)
                    target = min(8 * (si - base + 1) + 3, 39)
                    while done < target:
                        drive(g, 1)
                        done += 1
                        for a_ in aux:
                            next(a_, None)
                    next(sp_, None)
                    next(sp_, None)
                while done < 39:
                    drive(g, 1)
                    done += 1
                    for a_ in aux:
                        next(a_, None)
            else:
                for _ in range(39):
                    drive(g, 1)
                    for a_ in aux:
                        next(a_, None)
            for a_ in aux:
                for _ in a_:
                    pass
            prev = (pr, tt)
        if prev is not None:
            for _ in gen_normalize(*prev):
                pass
        if kstage < 4:
            emit_qkv_stripe(1, 0)
        for t_ in range(4):
            if t_ not in proj_done:
                for _ in gen_proj_tt(t_):
                    pass


@functools.cache
def _build():
    nc = bacc.Bacc("TRN2", target_bir_lowering=False, debug=False)
    xp = nc.dram_tensor("xp", [C, T], F32, kind="ExternalInput").ap()
    wqkv = nc.dram_tensor("wqkv", [3 * C, C], F32, kind="ExternalInput").ap()
    bqkv = nc.dram_tensor("bqkv", [3 * C, 1], F32, kind="ExternalInput").ap()
    wproj = nc.dram_tensor("wproj", [C, C], F32, kind="ExternalInput").ap()
    bproj = nc.dram_tensor("bproj", [C, 1], F32, kind="ExternalInput").ap()
    gamma = nc.dram_tensor("gamma", [C, 1], F32, kind="ExternalInput").ap()
    beta = nc.dram_tensor("beta", [C, 1], F32, kind="ExternalInput").ap()
    outp = nc.dram_tensor("outp", [C, TQ], F32, kind="ExternalOutput").ap()
    dbgo = nc.dram_tensor("dbgo", [P, 3, 512], F32, kind="ExternalOutput").ap()
    with tile.TileContext(nc) as tc:
        _emit(nc, tc, (xp, wqkv, bqkv, wproj, bproj, gamma, beta, outp, dbgo))
    nc.finalize()
    return nc


def _make_in_maps(x, gamma, beta, w_qkv, qkv_bias, w_proj, proj_bias):
    xf = np.ascontiguousarray(np.asarray(x, np.float32)).reshape(4, C, T)
    shared = {
        "wqkv": np.ascontiguousarray(np.asarray(w_qkv, np.float32)),
        "bqkv": np.ascontiguousarray(np.asarray(qkv_bias, np.float32).reshape(3 * C, 1)),
        "wproj": np.ascontiguousarray(np.asarray(w_proj, np.float32)),
        "bproj": np.ascontiguousarray(np.asarray(proj_bias, np.float32).reshape(C, 1)),
        "gamma": np.ascontiguousarray(np.asarray(gamma, np.float32).reshape(C, 1)),
        "beta": np.ascontiguousarray(np.asarray(beta, np.float32).reshape(C, 1)),
    }
    in_maps = []
    for core in range(N_CORES):
        b, half = divmod(core, 2)
        if half == 0:
            xpc = xf[b]
        else:
            xpc = np.concatenate([xf[b][:, TQ:], xf[b][:, :TQ]], axis=1)
        in_maps.append({"xp": np.ascontiguousarray(xpc), **shared})
    return in_maps


def _run(in_maps, **kwargs):
    nc = _build()
    return run_bass_kernel_spmd(nc, in_maps, core_ids=list(range(N_CORES)), **kwargs)


def kernel(x, gamma, beta, w_qkv, qkv_bias, w_proj, proj_bias, num_heads):
    assert int(num_heads) == 4
    in_maps = _make_in_maps(x, gamma, beta, w_qkv, qkv_bias, w_proj, proj_bias)
    res = _run(in_maps)
    out = np.empty((4, C, T), np.float32)
    for core in range(N_CORES):
        b, half = divmod(core, 2)
        out[b][:, half * TQ:(half + 1) * TQ] = res.results[core]["outp"]
    return out.reshape(4, C, 64, 64)
